# revision 17
# baseline (speedup 1.0000x reference)
"""Trainium2 Bass kernel for CausalSelfAttention with Mixture-of-Heads routing.

Sharding: tensor-parallel over heads across 8 cores. Core i computes Q heads
(2i, 2i+1) and KV head i, runs causal GQA attention for those heads, applies
the (replicated) top-8-of-16 head-router mask, and produces a rank-128 partial
of the output projection. The host sums the 8 partials.

On-device layout is "dims x tokens" (transposed activations): host supplies
x^T once; projections, rotary, RMS-norm, scores, and attn@V all run in this
layout, with PE transposes only for V (to token-major) and the head mask.
Matmuls run in fp32r (single-pass fp32, ~1e-4) except the router, which runs
in full fp32 so top-k decisions match an fp32 reference exactly.
"""
import sys

sys.path.insert(0, "/opt/trn_rl_repo")

import numpy as np

import concourse.bass as bass
import concourse.mybir as mybir
from concourse import bacc
from concourse.tile import TileContext
from concourse.bass_utils import run_bass_kernel_spmd

B, T, C = 4, 2048, 1024
NH, NKV, HD = 16, 8, 64
NT = B * T           # 8192 flattened tokens
N_CORES = 8
EPS = 1e-6
VG = 32
TC = 256             # phase-1 token chunk
QC = 512             # phase-2 q chunk
f32 = mybir.dt.float32
f32r = mybir.dt.float32r

_cache = {}


def _build():
    from concourse.hw_specs import get_activation_tables

    tables = get_activation_tables("gen3")
    if "natural_log_exp_and_others" in tables:
        for sname in ("exp_and_others", "exp_and_friends"):
            if sname in tables:
                tables[sname].clear()

    nc = bacc.Bacc("TRN2", target_bir_lowering=False, debug=False, num_devices=N_CORES)

    xT_d = nc.dram_tensor("xT", (C, NT), f32, kind="ExternalInput")
    wq_d = nc.dram_tensor("wqT", (C, 128), f32, kind="ExternalInput")
    wkv_d = nc.dram_tensor("wkvT", (C, 128), f32, kind="ExternalInput")
    wrg_d = nc.dram_tensor("wrgT", (C, 17), f32, kind="ExternalInput")
    wo_d = nc.dram_tensor("woT", (128, C), f32, kind="ExternalInput")
    cs_d = nc.dram_tensor("cossinT", (64, T), f32, kind="ExternalInput")
    ve_d = nc.dram_tensor("ve", (NT, HD), f32, kind="ExternalInput")
    tri_d = nc.dram_tensor("tri", (128, 128), f32, kind="ExternalInput")
    ident_d = nc.dram_tensor("ident", (128, 128), f32, kind="ExternalInput")
    oq_d = nc.dram_tensor("oq", (128, 33), f32, kind="ExternalInput")
    ok_d = nc.dram_tensor("okc", (64, 1), f32, kind="ExternalInput")
    vones_d = nc.dram_tensor("vones", (128, NT // 128, 1), f32, kind="ExternalInput")

    part_d = nc.dram_tensor("part", (NT, C), f32, kind="ExternalOutput")
    hm_d = nc.dram_tensor("hmask", (NT, NH), f32, kind="ExternalOutput")
    DEBUG = bool(__import__("os").environ.get("KDEBUG"))
    if DEBUG:
        dbg_q = nc.dram_tensor("dbg_q", (128, NT), f32, kind="ExternalOutput")
        dbg_k = nc.dram_tensor("dbg_k", (128, NT), f32, kind="ExternalOutput")
        dbg_v = nc.dram_tensor("dbg_v", (128, NT // 128, HD + 1), f32, kind="ExternalOutput")

    with TileContext(nc) as tc:
        with (
            tc.tile_pool(name="const", bufs=1) as cp,
            tc.tile_pool(name="persist", bufs=1) as pp,
            tc.tile_pool(name="w1", bufs=2) as w1,      # phase-1 working
            tc.tile_pool(name="w2", bufs=6) as w2,      # phase-2 exp tiles
            tc.tile_pool(name="w3", bufs=2) as w3,      # phase-2/3 misc
            tc.tile_pool(name="psA", bufs=4, space="PSUM") as psA,
            tc.tile_pool(name="psB", bufs=2, space="PSUM") as psB,
            tc.tile_pool(name="psC", bufs=2, space="PSUM") as psC,
        ):
            # ---------------- constants ----------------
            wq_s = cp.tile([128, 8, 128], f32r, tag="wq")
            nc.gpsimd.dma_start(out=wq_s, in_=wq_d.ap().rearrange("(k p) m -> p k m", p=128))
            wkv_s = cp.tile([128, 8, 128], f32r, tag="wkv")
            nc.gpsimd.dma_start(out=wkv_s, in_=wkv_d.ap().rearrange("(k p) m -> p k m", p=128))
            wrg_s = cp.tile([128, 8, 17], f32, tag="wrg")
            nc.sync.dma_start(out=wrg_s, in_=wrg_d.ap().rearrange("(k p) m -> p k m", p=128))
            wo_s = cp.tile([128, C], f32r, tag="wo")
            nc.gpsimd.dma_start(out=wo_s, in_=wo_d.ap())
            cs_s = cp.tile([64, T], f32, tag="cs")
            nc.sync.dma_start(out=cs_s, in_=cs_d.ap())
            tri_s = cp.tile([128, 128], f32r, tag="tri")
            nc.gpsimd.dma_start(out=tri_s, in_=tri_d.ap())
            ident = cp.tile([128, 128], f32r, tag="ident")
            nc.gpsimd.dma_start(out=ident, in_=ident_d.ap())
            oq = cp.tile([128, 33], f32r, tag="oq")
            nc.gpsimd.dma_start(out=oq, in_=oq_d.ap())
            ok = cp.tile([64, 1], f32r, tag="ok")
            nc.gpsimd.dma_start(out=ok, in_=ok_d.ap())
            epsb = cp.tile([128, 1], f32, tag="epsb")
            nc.vector.memset(epsb, EPS)

            # ---------------- persistent activations ----------------
            qhT = pp.tile([128, NT], f32r, tag="qhT")        # 2 heads x 64d, tokens
            khT = pp.tile([128, NT], f32r, tag="khT")  # k-hat^T duplicated in both halves
            vn_s = pp.tile([128, NT // 128, HD + 1], f32r, tag="vn")
            mask_all = pp.tile([128, NT // 128, NH], f32r, tag="mask")
            nc.gpsimd.dma_start(out=vn_s[:, :, HD : HD + 1], in_=vones_d.ap())

            # ---------------- phase 1: projections/router per token chunk ----
            for jt in range(NT // TC):
                t0 = jt * TC
                tb0 = t0 % T
                xr = w1.tile([128, 8, TC], f32r, tag="xr")
                nc.gpsimd.dma_start(
                    out=xr, in_=xT_d.ap()[:, t0 : t0 + TC].rearrange("(k p) n -> p k n", p=128)
                )
                xf = w1.tile([128, 8, TC], f32, tag="xf")
                nc.sync.dma_start(
                    out=xf, in_=xT_d.ap()[:, t0 : t0 + TC].rearrange("(k p) n -> p k n", p=128)
                )

                psqkv = psA.tile([128, 2, TC], f32, tag="bank1")
                psq = psqkv[:, 0, :]
                pskv = psqkv[:, 1, :]
                for k in range(8):
                    nc.tensor.matmul(psq, wq_s[:, k, :], xr[:, k, :], start=(k == 0), stop=(k == 7))
                for k in range(8):
                    nc.tensor.matmul(pskv, wkv_s[:, k, :], xr[:, k, :], start=(k == 0), stop=(k == 7))
                psmisc = psB.tile([128, 512], f32, tag="bank2")
                psr = psmisc[:, 0:34].rearrange("p (s m) -> p s m", s=TC // 128)
                for sub in range(TC // 128):
                    for k in range(8):
                        nc.tensor.matmul(
                            psr[:, sub, :],
                            xf[:, k, sub * 128 : (sub + 1) * 128],
                            wrg_s[:, k, :],
                            start=(k == 0),
                            stop=(k == 7),
                        )

                # rotary, in place in PSUM (dims x tokens layout), q (2 heads) + k
                cosv = cs_s[0:32, tb0 : tb0 + TC]
                sinv = cs_s[32:64, tb0 : tb0 + TC]
                rtmp = w1.tile([128, TC], f32, tag="rottmp")
                for psrc, nh in ((psq, 2), (pskv, 1)):
                    for h in range(nh):
                        x1 = psrc[64 * h : 64 * h + 32, :]       # PSUM
                        x2 = psrc[64 * h + 32 : 64 * h + 64, :]  # PSUM
                        t_a = rtmp[64 * h : 64 * h + 32, :]
                        t_b = rtmp[64 * h + 32 : 64 * h + 64, :]
                        nc.vector.tensor_mul(t_a, x2, sinv)
                        nc.vector.tensor_mul(t_b, x1, sinv)
                        nc.vector.tensor_mul(x1, x1, cosv)
                        nc.vector.tensor_add(x1, x1, t_a)
                        nc.vector.tensor_mul(x2, x2, cosv)
                        nc.vector.tensor_sub(x2, x2, t_b)

                # rms-norm scales via ones-matmul + ln/exp
                qsq = w1.tile([128, TC], f32r, tag="qsq")
                nc.scalar.activation(qsq, psq, mybir.ActivationFunctionType.Square)
                ksq = w1.tile([64, TC], f32r, tag="ksq")
                nc.scalar.activation(ksq, pskv[0:64, :], mybir.ActivationFunctionType.Square)
                pssum = psC.tile([128, 512], f32, tag="bank3")
                ssq = pssum[0:33, 0:TC]
                nc.tensor.matmul(ssq, oq, qsq, start=True, stop=True)
                ssk = pssum[0:1, TC : 2 * TC]
                nc.tensor.matmul(ssk, ok, ksq, start=True, stop=True)
                rinv = w1.tile([33, TC], f32, tag="rinv")
                nc.scalar.activation(rinv, ssq, mybir.ActivationFunctionType.Ln,
                                     bias=epsb[0:33], scale=1.0 / HD)
                nc.scalar.activation(rinv, rinv, mybir.ActivationFunctionType.Exp, scale=-0.5)
                rinvk = w1.tile([1, TC], f32, tag="rinvk")
                nc.scalar.activation(rinvk, ssk, mybir.ActivationFunctionType.Ln,
                                     bias=epsb[0:1], scale=1.0 / HD)
                nc.scalar.activation(rinvk, rinvk, mybir.ActivationFunctionType.Exp, scale=-0.5)
                rb = w1.tile([128, TC], f32, tag="rb")
                nc.gpsimd.partition_broadcast(rb[0:64, :], rinv[0:1, :])
                nc.gpsimd.partition_broadcast(rb[64:128, :], rinv[32:33, :])
                rkb = w1.tile([64, TC], f32, tag="rkb")
                nc.gpsimd.partition_broadcast(rkb, rinvk[0:1, :])
                nc.vector.tensor_mul(qhT[:, t0 : t0 + TC], qrot, rb)
                nc.vector.tensor_mul(khT[0:64, t0 : t0 + TC], krot, rkb)
                nc.vector.tensor_mul(khT[64:128, t0 : t0 + TC], krot, rkb)

                # v: transpose to token-major, add gated value-embedding
                vt = w1.tile([64, TC], f32r, tag="vt")
                nc.vector.tensor_copy(vt, pskv[64:128, :])
                for sub in range(TC // 128):
                    cv = t0 // 128 + sub
                    psv = psmisc[:, 34 + sub * 64 : 34 + (sub + 1) * 64].bitcast(f32r)
                    nc.tensor.transpose(psv, vt[:, sub * 128 : (sub + 1) * 128], ident[0:64, 0:64])
                    eg = w1.tile([128, 1], f32, tag="eg")
                    nc.scalar.activation(eg, psr[:, sub, 16:17],
                                         mybir.ActivationFunctionType.Exp, scale=-1.0)
                    nc.vector.tensor_scalar_add(eg, eg, 1.0)
                    nc.vector.reciprocal(out=eg, in_=eg)
                    ve_s = w1.tile([128, HD], f32, tag="ve")
                    nc.sync.dma_start(out=ve_s, in_=ve_d.ap()[cv * 128 : (cv + 1) * 128, :])
                    gev = w1.tile([128, HD], f32, tag="gev")
                    nc.vector.tensor_scalar(
                        out=gev, in0=ve_s, scalar1=eg[:, 0:1], scalar2=2.0,
                        op0=mybir.AluOpType.mult, op1=mybir.AluOpType.mult,
                    )
                    nc.vector.tensor_add(vn_s[:, cv, 0:HD], psv, gev)

                    # router mask: top-8 of 16 (>= kth max), store + transpose
                    lgs = w1.tile([128, 16], f32, tag="lgs")
                    nc.vector.tensor_copy(lgs, psr[:, sub, 0:16])
                    mx = w1.tile([128, 8], f32, tag="mx")
                    nc.vector.max(out=mx, in_=lgs)
                    nc.vector.tensor_scalar(
                        out=mask_all[:, cv, :], in0=lgs, scalar1=mx[:, 7:8], scalar2=None,
                        op0=mybir.AluOpType.is_ge,
                    )

            nc.sync.dma_start(
                out=hm_d.ap().rearrange("(c p) h -> p c h", p=128),
                in_=mask_all.bitcast(f32),
            )

            if DEBUG:
                nc.sync.dma_start(out=dbg_q.ap(), in_=qhT.bitcast(f32))
                nc.sync.dma_start(out=dbg_k.ap(), in_=khT.bitcast(f32))
                nc.sync.dma_start(out=dbg_v.ap(), in_=vn_s.bitcast(f32))

            # ------------- phase 2+3: attention, mask/normalize, out-proj ----
            for b in range(B):
                for jq in range(T // QC):
                    qg0 = b * T + jq * QC
                    n_sc = (jq + 1) * (QC // 128)
                    y2 = w3.tile([128, QC], f32r, tag="y2")
                    for h in range(2):
                        y_full = psB.tile([128, 512], f32, tag="bank2")
                        y_acc = y_full[0 : HD + 1, :]
                        for isc in range(n_sc):
                            sg0 = b * T + isc * 128
                            flo = max(0, isc * 128 - jq * QC)
                            sc_ps = psA.tile([128, QC], f32, tag="bank1")
                            nc.tensor.matmul(
                                sc_ps[:, flo:QC],
                                khT[64 * h : 64 * h + 64, sg0 : sg0 + 128],
                                qhT[64 * h : 64 * h + 64, qg0 + flo : qg0 + QC],
                                start=True, stop=True,
                            )
                            et = w2.tile([128, QC], f32r, tag="et")
                            nc.scalar.activation(
                                et[:, flo:QC], sc_ps[:, flo:QC],
                                mybir.ActivationFunctionType.Exp, scale=0.125,
                            )
                            if isc * 128 >= jq * QC:  # diagonal: triangular mask
                                nc.vector.tensor_mul(
                                    et[:, flo : flo + 128], et[:, flo : flo + 128], tri_s
                                )
                            nc.tensor.matmul(
                                y_acc[:, flo:QC],
                                vn_s[:, b * (T // 128) + isc, :],
                                et[:, flo:QC],
                                start=(isc == 0), stop=(isc == n_sc - 1),
                            )
                        rr = w3.tile([1, QC], f32, tag="rr")
                        nc.vector.reciprocal(out=rr, in_=y_acc[HD : HD + 1, :])
                        psmT = psC.tile([128, 512], f32, tag="bank3")
                        for m in range(QC // 128):
                            cg = (b * T + jq * QC) // 128 + m
                            nc.tensor.transpose(
                                psmT[0:1, m * 128 : (m + 1) * 128].bitcast(f32r),
                                mask_all[:, cg, h : h + 1],
                                ident,
                            )
                        mrow = w3.tile([1, QC], f32, tag="mrow")
                        nc.vector.tensor_copy(mrow, psmT[0:1, :])
                        sc_t = w3.tile([1, QC], f32, tag="sct")
                        nc.vector.tensor_mul(sc_t, rr, mrow)
                        scb = w3.tile([64, QC], f32, tag="scb")
                        nc.gpsimd.partition_broadcast(scb, sc_t[0:1, :])
                        nc.vector.tensor_mul(y2[64 * h : 64 * h + 64, :], y_acc[0:HD, :], scb)

                    for sub in range(QC // 128):
                        tok0 = qg0 + sub * 128
                        oc = w3.tile([128, C], f32, tag="oc")
                        for half in range(2):
                            pso = psA.tile([128, 512], f32, tag="bank1")
                            nc.tensor.matmul(
                                pso, y2[:, sub * 128 : (sub + 1) * 128],
                                wo_s[:, half * 512 : (half + 1) * 512], start=True, stop=True,
                            )
                            if half == 0:
                                nc.vector.tensor_copy(oc[:, 0:512], pso)
                            else:
                                nc.scalar.copy(out=oc[:, 512:1024], in_=pso)
                        nc.sync.dma_start(out=part_d.ap()[tok0 : tok0 + 128, :], in_=oc)

    nc.compile()
    return nc


def _get_nc():
    if "nc" not in _cache:
        _cache["nc"] = _build()
    return _cache["nc"]


def kernel(x, ve, cos, sin, Wq, Wk, Wv, Wo, Wr, Wg, window_size):
    x = np.asarray(x, np.float32)
    ve = np.asarray(ve, np.float32)
    cos = np.asarray(cos, np.float32)
    sin = np.asarray(sin, np.float32)
    Wq = np.asarray(Wq, np.float32)
    Wk = np.asarray(Wk, np.float32)
    Wv = np.asarray(Wv, np.float32)
    Wo = np.asarray(Wo, np.float32)
    Wr = np.asarray(Wr, np.float32)
    Wg = np.asarray(Wg, np.float32)

    nc = _get_nc()

    xT = np.ascontiguousarray(x.reshape(NT, C).T)
    cosT = np.ascontiguousarray(cos[0, :, 0, :].T)   # (32, T)
    sinT = np.ascontiguousarray(sin[0, :, 0, :].T)
    cossinT = np.concatenate([cosT, sinT], axis=0)   # (64, T)
    tri = (np.arange(128)[:, None] <= np.arange(128)[None, :]).astype(np.float32)
    ident = np.eye(128, dtype=np.float32)
    oq_c = np.zeros((128, 33), np.float32)
    oq_c[0:64, 0] = 1.0
    oq_c[64:128, 32] = 1.0
    ok_c = np.ones((64, 1), np.float32)
    vones = np.ones((128, 64, 1), np.float32)
    ve_r = ve.reshape(NT, NKV, HD)

    perms = []
    in_maps = []
    for i in range(N_CORES):
        perm = [2 * i, 2 * i + 1] + [h for h in range(NH) if h not in (2 * i, 2 * i + 1)]
        perms.append(perm)
        wg_pad = np.zeros((1, C), np.float32)
        wg_pad[0, :VG] = Wg[i]
        wrg = np.concatenate([Wr[perm], wg_pad], axis=0)           # (17, C)
        in_maps.append({
            "xT": xT,
            "wqT": np.ascontiguousarray(Wq[128 * i : 128 * (i + 1)].T),
            "wkvT": np.ascontiguousarray(
                np.concatenate([Wk[64 * i : 64 * (i + 1)], Wv[64 * i : 64 * (i + 1)]], 0).T
            ),
            "wrgT": np.ascontiguousarray(wrg.T),
            "woT": np.ascontiguousarray(Wo[:, 128 * i : 128 * (i + 1)].T),
            "cossinT": cossinT,
            "ve": np.ascontiguousarray(ve_r[:, i, :]),
            "tri": tri,
            "ident": ident,
            "oq": oq_c,
            "okc": ok_c,
            "vones": vones,
        })

    res = run_bass_kernel_spmd(nc, in_maps, core_ids=list(range(N_CORES)))

    out = np.zeros((NT, C), np.float64)
    for i in range(N_CORES):
        out += res.results[i]["part"]
    out = out.astype(np.float32).reshape(B, T, C)

    hm = np.empty((NT, NH), np.float32)
    hm[:, perms[0]] = res.results[0]["hmask"]
    hm = hm.reshape(B, T, NH)
    return out, hm
